# revision 5
# baseline (speedup 1.0000x reference)
"""Trainium2 kernel for nn_Classifier_65111704207481 (retrieval_knn).

reference computes, for Z,Y [8192, 2048] f32:
    sim[i,j] = (y_i . x_j) / max(|y_i||x_j|, 1e-8)   (x=Z rows, y=Y rows)
    top1  = mean(argmax(sim, axis=1) == arange)
    top10 = mean(arange in top_k(sim, 10))
returns (top1, top10, sim).

Strategy: rows of Y (and of sim) are sharded across 8 NeuronCores.
The host pre-normalizes rows (so the device GEMM directly produces sim),
lays both operands out feature-major ([128 k-partitions, k-tile, sample])
and each core computes its [1024, 8192] slab as a PE matmul streamed over
16 column chunks of 512. Top-k accuracy is reduced on-device to one
count per row: cnt_i = #{j : sim[i,j] > sim[i,i]} (diagonal extracted
bit-exactly from the slab itself via an identity-mask reduce), so
    hit@1 = (cnt == 0), hit@10 = (cnt <= 9).
The host just means those bits and concatenates the slabs.

MODE:
  "f32r"   - single full-rate matmul stream in fp32r (e8m11). ~3x faster.
  "bf16x3" - hi/lo bf16 split, 3 accumulating matmuls (fp32-grade error).
  "f32"    - native fp32 matmuls (4 cycles/row, slowest, exact).
"""

import sys

for _p in ("/opt/trn_rl_repo", "/root/.axon_site/_ro/trn_rl_repo"):
    if _p not in sys.path:
        sys.path.append(_p)

import numpy as np

MODE = "f32r"

B, F = 8192, 2048
NCORES = 8
BL = B // NCORES  # 1024 rows of sim per core
P = 128  # partitions
KT = F // P  # 16 k-tiles
NW = 512  # column-chunk width (one PSUM bank)
NCH = B // NW  # 16 column chunks
MT = BL // P  # 8 row tiles per core

_compiled = {}


def _build(mode, features=("diag", "count")):
    import concourse.bacc as bacc
    import concourse.mybir as mybir
    from concourse.tile import TileContext
    from concourse.masks import make_identity

    dt = mybir.dt
    nc = bacc.Bacc("TRN2", target_bir_lowering=False, debug=False,
                   num_devices=NCORES)

    if mode == "bf16x3":
        in_specs = [("yTh", [P, KT, BL], dt.bfloat16),
                    ("yTl", [P, KT, BL], dt.bfloat16),
                    ("xTh", [NCH, P, KT, NW], dt.bfloat16),
                    ("xTl", [NCH, P, KT, NW], dt.bfloat16)]
    else:
        mmdt = dt.float32r if mode == "f32r" else dt.float32
        in_specs = [("yT", [P, KT, BL], mmdt),
                    ("xT", [NCH, P, KT, NW], mmdt)]

    aps = {nm: nc.dram_tensor(nm, shp, d, kind="ExternalInput").ap()
           for nm, shp, d in in_specs}
    sim_ap = nc.dram_tensor("SIM", [BL, B], dt.float32, kind="ExternalOutput").ap()
    cnt_ap = nc.dram_tensor("CNT", [P, MT], dt.float32, kind="ExternalOutput").ap()

    with TileContext(nc) as tc:
        with (
            tc.tile_pool(name="ypool", bufs=1) as ypool,
            tc.tile_pool(name="xpool", bufs=3) as xpool,
            tc.tile_pool(name="psum", bufs=8, space="PSUM") as psum,
            tc.tile_pool(name="spool", bufs=4) as spool,
            tc.tile_pool(name="jpool", bufs=4) as jpool,
            tc.tile_pool(name="small", bufs=1) as small,
        ):
            # resident stationary operand (the local y rows, feature-major)
            ytiles = {}
            for nm, shp, d in in_specs:
                if nm.startswith("y"):
                    t = small.tile(shp, d, name=nm)
                    nc.sync.dma_start(out=t, in_=aps[nm])
                    ytiles[nm] = t

            ident = small.tile([P, P], dt.float32, name="ident")
            make_identity(nc, ident)

            d_tiles = [small.tile([P, 1], dt.float32, name=f"d{m}")
                       for m in range(MT)]
            cnt_tiles = [small.tile([P, NCH], dt.float32, name=f"cnt{m}")
                         for m in range(MT)]

            # partition_id is not needed: each core's in_map carries its own
            # y rows, so the same program runs unchanged on all cores. The
            # diagonal of the global sim lands in a per-core fixed column
            # band only if the host rotates the column chunks per core;
            # instead the host passes x chunks pre-rotated so that chunk 0
            # always holds this core's diagonal columns (see _prep below).
            x_loaded = {}

            def load_chunk(j):
                tiles = []
                for nm, shp, d in in_specs:
                    if nm.startswith("x"):
                        t = xpool.tile([P, KT, NW], d, tag=nm)
                        nc.sync.dma_start(out=t, in_=aps[nm][j])
                        tiles.append((nm, t))
                x_loaded[j] = tiles
                return tiles

            def mm_block(xt, m, pt):
                if mode == "bf16x3":
                    yh, yl = ytiles["yTh"], ytiles["yTl"]
                    xh = dict(xt)["xTh"]
                    xl = dict(xt)["xTl"]
                    n = KT * 3
                    i = 0
                    for kt in range(KT):
                        for lhs, rhs in ((yh, xh), (yh, xl), (yl, xh)):
                            nc.tensor.matmul(
                                pt, lhsT=lhs[:, kt, m * P:(m + 1) * P],
                                rhs=rhs[:, kt, :],
                                start=(i == 0), stop=(i == n - 1))
                            i += 1
                else:
                    y = ytiles["yT"]
                    x = dict(xt)["xT"]
                    for kt in range(KT):
                        nc.tensor.matmul(
                            pt, lhsT=y[:, kt, m * P:(m + 1) * P],
                            rhs=x[:, kt, :],
                            start=(kt == 0), stop=(kt == KT - 1))

            def process(j, m, extract_diag):
                xt = x_loaded[j]
                pt = psum.tile([P, NW], mybir.dt.float32, name="pt")
                mm_block(xt, m, pt)
                st = spool.tile([P, NW], mybir.dt.float32, name="st")
                nc.vector.tensor_copy(st, pt)
                nc.sync.dma_start(
                    out=sim_ap[m * P:(m + 1) * P, j * NW:(j + 1) * NW], in_=st)
                if extract_diag and "diag" in features:
                    off = (m * P) % NW
                    jt128 = jpool.tile([P, P], mybir.dt.float32, tag="j128")
                    nc.vector.tensor_mul(jt128, st[:, off:off + P], ident)
                    nc.vector.reduce_sum(d_tiles[m], jt128,
                                         axis=mybir.AxisListType.X)
                if "count" not in features:
                    return
                jt = jpool.tile([P, NW], mybir.dt.float32, tag="jfull")
                nc.vector.tensor_scalar(
                    out=jt, in0=st, scalar1=d_tiles[m], scalar2=None,
                    op0=mybir.AluOpType.is_gt, op1=mybir.AluOpType.add,
                    accum_out=cnt_tiles[m][:, j:j + 1])

            # The host rotates x chunks per core so that device chunks
            # 0..ndc-1 hold this core's diagonal columns: m-tile m's diag
            # sits in chunk m // mpc at offset (m*P) % NW. Those chunks go
            # first so every row's diagonal value d is extracted before the
            # rest of that row streams through the count op.
            mpc = NW // P  # m-tiles per chunk
            ndc = MT // mpc  # chunks covering the diagonal band
            assert MT == ndc * mpc
            for dc in range(ndc):
                load_chunk(dc)
            for dc in range(ndc):
                for m in range(dc * mpc, (dc + 1) * mpc):
                    process(dc, m, True)
            if ndc < NCH:
                load_chunk(ndc)
            for dc in range(ndc):
                for m in range(MT):
                    if m // mpc != dc:
                        process(dc, m, False)
            for j in range(ndc, NCH):
                if j + 1 < NCH:
                    load_chunk(j + 1)
                for m in range(MT):
                    process(j, m, False)

            cfin = small.tile([P, MT], mybir.dt.float32, name="cfin")
            if "count" in features:
                for m in range(MT):
                    nc.vector.reduce_sum(cfin[:, m:m + 1], cnt_tiles[m],
                                         axis=mybir.AxisListType.X)
            else:
                nc.vector.memset(cfin, 0.0)
            nc.sync.dma_start(out=cnt_ap, in_=cfin)

    nc.compile()
    return nc


FEATURES = ("diag", "count")


def _get_nc(mode):
    key = (mode, FEATURES)
    if key not in _compiled:
        _compiled[key] = _build(mode, FEATURES)
    return _compiled[key]


def _prep(Z, Y, mode):
    """Host-side: normalize rows, build feature-major tiled layouts."""
    import ml_dtypes

    x = np.asarray(Z, dtype=np.float32).reshape(B, F)
    y = np.asarray(Y, dtype=np.float32).reshape(B, F)
    xn = np.sqrt(np.einsum("ij,ij->i", x, x, dtype=np.float64))
    yn = np.sqrt(np.einsum("ij,ij->i", y, y, dtype=np.float64))
    xh = (x / xn[:, None]).astype(np.float32)
    yh = (y / yn[:, None]).astype(np.float32)

    # xT_global[ch, p, kt, n] = xh[512*ch + n, 128*kt + p]
    xT = np.ascontiguousarray(
        xh.reshape(NCH, NW, KT, P).transpose(0, 3, 2, 1))

    in_maps = []
    for c in range(NCORES):
        yc = yh[c * BL:(c + 1) * BL]  # [1024, 2048]
        yT = np.ascontiguousarray(yc.reshape(BL, KT, P).transpose(2, 1, 0))
        # rotate chunks so chunk 0..1 hold this core's diagonal columns
        rot = np.roll(np.arange(NCH), -(BL // NW) * c)
        xTc = np.ascontiguousarray(xT[rot])
        if mode == "bf16x3":
            yTh = yT.astype(ml_dtypes.bfloat16)
            yTl = (yT - yTh.astype(np.float32)).astype(ml_dtypes.bfloat16)
            xTh = xTc.astype(ml_dtypes.bfloat16)
            xTl = (xTc - xTh.astype(np.float32)).astype(ml_dtypes.bfloat16)
            in_maps.append({"yTh": yTh, "yTl": yTl, "xTh": xTh, "xTl": xTl})
        else:
            in_maps.append({"yT": yT, "xT": xTc})
    return in_maps


def run(Z, Y, mode=None, trace=False, trace_cores=None):
    from concourse.bass_utils import run_bass_kernel_spmd

    mode = mode or MODE
    nc = _get_nc(mode)
    in_maps = _prep(Z, Y, mode)
    kw = {}
    if trace:
        kw = dict(trace=True,
                  trace_cores=trace_cores or list(range(NCORES)))
    res = run_bass_kernel_spmd(nc, in_maps, core_ids=list(range(NCORES)), **kw)

    sim_parts = []
    counts = np.empty(B, dtype=np.int64)
    for c in range(NCORES):
        out = res.results[c]
        # undo the per-core chunk rotation of the columns
        simc = out["SIM"].reshape(BL, NCH, NW)
        rot = np.roll(np.arange(NCH), -(BL // NW) * c)
        inv = np.argsort(rot)
        sim_parts.append(simc[:, inv].reshape(BL, B))
        cntc = out["CNT"]  # [P, MT], row (local) = m*128 + p
        counts[c * BL:(c + 1) * BL] = \
            np.rint(cntc.T.reshape(BL)).astype(np.int64)
    sim = np.concatenate(sim_parts, axis=0)
    top1 = np.float32(np.mean((counts == 0).astype(np.float32)))
    top10 = np.float32(np.mean((counts <= 9).astype(np.float32)))
    return (top1, top10, sim), res


def kernel(Z, Y):
    out, _ = run(Z, Y)
    return out


# revision 15
# speedup vs baseline: 2.7171x; 2.7171x over previous
"""Trainium2 kernel for nn_Classifier_65111704207481 (retrieval_knn).

reference computes, for Z,Y [8192, 2048] f32:
    sim[i,j] = (y_i . x_j) / max(|y_i||x_j|, 1e-8)   (x=Z rows, y=Y rows)
    top1  = mean(argmax(sim, axis=1) == arange)
    top10 = mean(arange in top_k(sim, 10))
returns (top1, top10, sim).

Strategy: rows of Y (and of sim) are sharded across 8 NeuronCores.
The host pre-normalizes rows (so the device GEMM directly produces sim),
lays both operands out feature-major ([128 k-partitions, k-tile, sample])
and each core computes its [1024, 8192] slab as a PE matmul streamed over
16 column chunks of 512. Top-k accuracy is reduced on-device to one
count per row: cnt_i = #{j : sim[i,j] > sim[i,i]} (diagonal extracted
bit-exactly from the slab itself via an identity-mask reduce), so
    hit@1 = (cnt == 0), hit@10 = (cnt <= 9).
The host just means those bits and concatenates the slabs.

MODE:
  "f32r"   - single full-rate matmul stream in fp32r (e8m11). ~3x faster.
  "bf16x3" - hi/lo bf16 split, 3 accumulating matmuls (fp32-grade error).
  "f32"    - native fp32 matmuls (4 cycles/row, slowest, exact).
"""

import sys

for _p in ("/opt/trn_rl_repo", "/root/.axon_site/_ro/trn_rl_repo"):
    if _p not in sys.path:
        sys.path.append(_p)

import numpy as np

MODE = "f32r"

B, F = 8192, 2048
NCORES = 8
BL = B // NCORES  # 1024 rows of sim per core
P = 128  # partitions
KT = F // P  # 16 k-tiles
NW = 512  # column-chunk width (one PSUM bank)
NCH = B // NW  # 16 column chunks
MT = BL // P  # 8 row tiles per core

_compiled = {}


LDW_OPT = False


def _patch_ldw_opt():
    import concourse.bass_utils as bu
    if getattr(bu, "_ldw_patched", False):
        return
    orig = bu.run_command

    def run_command_ldw(cmd, *a, **k):
        cmd = ["--enable-ldw-opt=true" if c == "--enable-ldw-opt=false" else c
               for c in cmd]
        return orig(cmd, *a, **k)

    bu.run_command = run_command_ldw
    bu._ldw_patched = True


def _build(mode, features=("diag", "count")):
    import concourse.bacc as bacc
    import concourse.mybir as mybir
    from concourse.tile import TileContext
    from concourse.masks import make_identity

    if LDW_OPT:
        _patch_ldw_opt()
    dt = mybir.dt
    nc = bacc.Bacc("TRN2", target_bir_lowering=False, debug=False,
                   num_devices=NCORES)

    if mode == "bf16x3":
        in_specs = [("yTh", [P, KT, BL], dt.bfloat16),
                    ("yTl", [P, KT, BL], dt.bfloat16),
                    ("xTh", [NCH, P, KT, NW], dt.bfloat16),
                    ("xTl", [NCH, P, KT, NW], dt.bfloat16)]
    else:
        mmdt = dt.float32r if mode == "f32r" else dt.float32
        in_specs = [("yT", [P, KT, BL], mmdt),
                    ("xT", [NCH, P, KT, NW], mmdt)]

    aps = {nm: nc.dram_tensor(nm, shp, d, kind="ExternalInput").ap()
           for nm, shp, d in in_specs}
    sim_ap = nc.dram_tensor("SIM", [BL, B], dt.float32, kind="ExternalOutput").ap()
    cnt_ap = nc.dram_tensor("CNT", [P, MT], dt.float32, kind="ExternalOutput").ap()

    with TileContext(nc) as tc:
        with (
            tc.tile_pool(name="ypool", bufs=1) as ypool,
            tc.tile_pool(name="xpool", bufs=3) as xpool,
            tc.tile_pool(name="psum", bufs=8, space="PSUM") as psum,
            tc.tile_pool(name="spool", bufs=8) as spool,
            tc.tile_pool(name="jpool", bufs=4) as jpool,
            tc.tile_pool(name="small", bufs=1) as small,
        ):
            # resident stationary operand (the local y rows, feature-major).
            # One tile per k-tile, loaded individually, so the first matmul
            # group starts as soon as k-tile 0 lands instead of after the
            # whole 8MB, with exact DMA->matmul dependencies.
            ytiles = {}
            for nm, shp, d in in_specs:
                if nm.startswith("y"):
                    ytiles[nm] = [small.tile([P, BL], d, name=f"{nm}_{kt}")
                                  for kt in range(KT)]

            def load_y(kt):
                for nm, ts in ytiles.items():
                    nc.scalar.dma_start(out=ts[kt], in_=aps[nm][:, kt, :])

            ident = small.tile([P, P], dt.float32, name="ident")
            make_identity(nc, ident)

            d_tiles = [small.tile([P, 1], dt.float32, name=f"d{m}")
                       for m in range(MT)]
            cnt_tiles = [small.tile([P, NCH], dt.float32, name=f"cnt{m}")
                         for m in range(MT)]

            # partition_id is not needed: each core's in_map carries its own
            # y rows, so the same program runs unchanged on all cores. The
            # diagonal of the global sim lands in a per-core fixed column
            # band only if the host rotates the column chunks per core;
            # instead the host passes x chunks pre-rotated so that chunk 0
            # always holds this core's diagonal columns (see _prep below).
            x_loaded = {}

            def load_chunk(j, split=False):
                tiles = []
                for nm, shp, d in in_specs:
                    if nm.startswith("x"):
                        ts = [xpool.tile([P, NW], d, tag=f"{nm}{kt}",
                                         name=f"{nm}{kt}")
                              for kt in range(KT)]
                        tiles.append((nm, ts))
                x_loaded[j] = tiles
                if not split:
                    for kt in range(KT):
                        load_chunk_kt(j, kt)
                return tiles

            def load_chunk_kt(j, kt):
                for (nm, ts) in x_loaded[j]:
                    nc.sync.dma_start(out=ts[kt], in_=aps[nm][j][:, kt, :])

            def mm_one(xt, m, kt, pt):
                """One k-step of the accumulation group for (chunk, m)."""
                msl = slice(m * P, (m + 1) * P)
                if mode == "bf16x3":
                    yh, yl = ytiles["yTh"], ytiles["yTl"]
                    xh = dict(xt)["xTh"]
                    xl = dict(xt)["xTl"]
                    for ti, (lhs, rhs) in enumerate(
                            ((yh[kt], xh[kt]), (yh[kt], xl[kt]),
                             (yl[kt], xh[kt]))):
                        nc.tensor.matmul(
                            pt, lhsT=lhs[:, msl], rhs=rhs,
                            start=(kt == 0 and ti == 0),
                            stop=(kt == KT - 1 and ti == 2))
                else:
                    nc.tensor.matmul(
                        pt, lhsT=ytiles["yT"][kt][:, msl],
                        rhs=dict(xt)["xT"][kt],
                        start=(kt == 0), stop=(kt == KT - 1))

            def mm_block(xt, m, pt):
                for kt in range(KT):
                    mm_one(xt, m, kt, pt)

            def do_diag(m, st):
                off = (m * P) % NW
                jt128 = jpool.tile([P, P], mybir.dt.float32, tag="j128")
                nc.vector.tensor_mul(jt128, st[:, off:off + P], ident)
                nc.vector.reduce_sum(d_tiles[m], jt128,
                                     axis=mybir.AxisListType.X)

            def do_count(j, m, st):
                jt = jpool.tile([P, NW], mybir.dt.float32, tag="jfull")
                nc.vector.tensor_scalar(
                    out=jt, in0=st, scalar1=d_tiles[m], scalar2=None,
                    op0=mybir.AluOpType.is_gt, op1=mybir.AluOpType.add,
                    accum_out=cnt_tiles[m][:, j:j + 1])

            def drain_block(j, m, pt, extract_diag, count_now):
                st = spool.tile([P, NW], mybir.dt.float32, name="st")
                nc.vector.tensor_copy(st, pt)
                nc.sync.dma_start(
                    out=sim_ap[m * P:(m + 1) * P, j * NW:(j + 1) * NW], in_=st)
                if extract_diag and "diag" in features:
                    do_diag(m, st)
                if "count" in features:
                    if count_now:
                        do_count(j, m, st)
                    else:
                        deferred.append((j, m, st))

            def process(j, m, extract_diag=False, count_now=True):
                pt = psum.tile([P, NW], mybir.dt.float32, name="pt", tag="pt")
                mm_block(x_loaded[j], m, pt)
                drain_block(j, m, pt, extract_diag, count_now)

            # The host rotates x chunks per core so that device chunks
            # 0..ndc-1 hold this core's diagonal columns: m-tile m's diag
            # sits in chunk m // mpc at offset (m*P) % NW. Those chunks go
            # first so every row's diagonal value d is extracted before the
            # rest of that row streams through the count op.
            mpc = NW // P  # m-tiles per chunk
            ndc = MT // mpc  # chunks covering the diagonal band
            assert MT == ndc * mpc
            deferred = []
            for dc in range(ndc):
                load_chunk(dc, split=True)
            # interleave y and first-chunk loads k-tile by k-tile so the
            # PE's first accumulation group is fed at DMA pace
            for kt in range(KT):
                load_y(kt)
                load_chunk_kt(0, kt)
            for dc in range(1, ndc):
                for kt in range(KT):
                    load_chunk_kt(dc, kt)
            # Diag chunks run kt-major across all MT psum banks: the PE can
            # then consume each k-tile against every row group the moment
            # its DMA lands, instead of idling for a full chunk load.
            for dc in range(ndc):
                pts = [psum.tile([P, NW], mybir.dt.float32, name=f"pt{m}", tag="pt")
                       for m in range(MT)]
                for kt in range(KT):
                    for m in range(MT):
                        mm_one(x_loaded[dc], m, kt, pts[m])
                if dc + 1 == ndc and ndc < NCH:
                    load_chunk(ndc)
                for m in range(MT):
                    in_band = (m // mpc == dc)
                    d_ready = (m // mpc <= dc)
                    drain_block(dc, m, pts[m], in_band, d_ready)
            for (j, m, st) in deferred:
                do_count(j, m, st)
            for j in range(ndc, NCH):
                if j + 1 < NCH:
                    load_chunk(j + 1)
                for m in range(MT):
                    process(j, m)

            cfin = small.tile([P, MT], mybir.dt.float32, name="cfin")
            if "count" in features:
                for m in range(MT):
                    nc.vector.reduce_sum(cfin[:, m:m + 1], cnt_tiles[m],
                                         axis=mybir.AxisListType.X)
            else:
                nc.vector.memset(cfin, 0.0)
            nc.sync.dma_start(out=cnt_ap, in_=cfin)

    nc.compile()
    return nc


FEATURES = ("diag", "count")


def _get_nc(mode):
    key = (mode, FEATURES)
    if key not in _compiled:
        _compiled[key] = _build(mode, FEATURES)
    return _compiled[key]


def _prep(Z, Y, mode):
    """Host-side: normalize rows, build feature-major tiled layouts."""
    import ml_dtypes

    x = np.asarray(Z, dtype=np.float32).reshape(B, F)
    y = np.asarray(Y, dtype=np.float32).reshape(B, F)
    xn = np.sqrt(np.einsum("ij,ij->i", x, x, dtype=np.float64))
    yn = np.sqrt(np.einsum("ij,ij->i", y, y, dtype=np.float64))
    xh = (x / xn[:, None]).astype(np.float32)
    yh = (y / yn[:, None]).astype(np.float32)

    # xT_global[ch, p, kt, n] = xh[512*ch + n, 128*kt + p]
    xT = np.ascontiguousarray(
        xh.reshape(NCH, NW, KT, P).transpose(0, 3, 2, 1))

    in_maps = []
    for c in range(NCORES):
        yc = yh[c * BL:(c + 1) * BL]  # [1024, 2048]
        yT = np.ascontiguousarray(yc.reshape(BL, KT, P).transpose(2, 1, 0))
        # rotate chunks so chunk 0..1 hold this core's diagonal columns
        rot = np.roll(np.arange(NCH), -(BL // NW) * c)
        xTc = np.ascontiguousarray(xT[rot])
        if mode == "bf16x3":
            yTh = yT.astype(ml_dtypes.bfloat16)
            yTl = (yT - yTh.astype(np.float32)).astype(ml_dtypes.bfloat16)
            xTh = xTc.astype(ml_dtypes.bfloat16)
            xTl = (xTc - xTh.astype(np.float32)).astype(ml_dtypes.bfloat16)
            in_maps.append({"yTh": yTh, "yTl": yTl, "xTh": xTh, "xTl": xTl})
        else:
            in_maps.append({"yT": yT, "xT": xTc})
    return in_maps


def run(Z, Y, mode=None, trace=False, trace_cores=None):
    from concourse.bass_utils import run_bass_kernel_spmd

    mode = mode or MODE
    nc = _get_nc(mode)
    in_maps = _prep(Z, Y, mode)
    kw = {}
    if trace:
        kw = dict(trace=True,
                  trace_cores=trace_cores or list(range(NCORES)))
    res = run_bass_kernel_spmd(nc, in_maps, core_ids=list(range(NCORES)), **kw)

    sim_parts = []
    counts = np.empty(B, dtype=np.int64)
    for c in range(NCORES):
        out = res.results[c]
        # undo the per-core chunk rotation of the columns
        simc = out["SIM"].reshape(BL, NCH, NW)
        rot = np.roll(np.arange(NCH), -(BL // NW) * c)
        inv = np.argsort(rot)
        sim_parts.append(simc[:, inv].reshape(BL, B))
        cntc = out["CNT"]  # [P, MT], row (local) = m*128 + p
        counts[c * BL:(c + 1) * BL] = \
            np.rint(cntc.T.reshape(BL)).astype(np.int64)
    sim = np.concatenate(sim_parts, axis=0)
    top1 = np.float32(np.mean((counts == 0).astype(np.float32)))
    top10 = np.float32(np.mean((counts <= 9).astype(np.float32)))
    return (top1, top10, sim), res


def kernel(Z, Y):
    out, _ = run(Z, Y)
    return out
